# revision 14
# baseline (speedup 1.0000x reference)
"""MoE routing kernel for TRN2 (8 NeuronCores).

The reference MoE applies row 0's top-2 expert choice (indices and softmax
weights) to the entire batch, so the whole module collapses to

    out = x @ (w0*We[i0] + w1*We[i1]).T + (w0*be[i0] + w1*be[i1])

a single [16384,2048] @ [2048,2048] matmul with bias. Host does the tiny
row-0 gating and combines the two selected experts; the device runs the
matmul data-parallel over tokens (2048 tokens per core, no collectives).

v3 (bf16, dual-ring chase): x and W are cast to bf16 on host (fp32 PSUM
accumulation keeps rel err ~2e-3, far inside the 2e-2 gate), halving HBM
input traffic so the whole x shard is SBUF-resident before stage 2.
HWDGE issue cost is ~600ns per dma_start regardless of size, so inputs
move as few large chunks; only the first j-chunk of xp/W is split into
[128,512] kk-pieces (own tiles => slice-granular deps) to start the PE
~1.5us after the fixed ~7us NEFF preamble. xp and early xq ride the
Activation ring in parallel with W on the Sync ring to double issue
throughput during the quarter-0 chase. Schedule:
  Stage 1: m0..3 run k-outer in four n-quarters (4 PSUM banks each,
           ping-pong between bank sets), chasing the W stream.
  Stage 2: m4..15 run k-inner against resident W/x, banks rotating mod 8.
Evictions (PSUM + bias -> SBUF -> HBM) trail on DVE/Activation queues.
"""

import os
import sys

import numpy as np

if "/opt/trn_rl_repo" not in sys.path:
    sys.path.insert(0, "/opt/trn_rl_repo")

N, D, E, TOPK = 16384, 2048, 8, 2
N_CORES = 8
P = 128
M_SHARD = N // N_CORES  # 2048 tokens per core
K_TILES = D // P        # 16 contraction slabs
M_TILES = M_SHARD // P  # 16
N_FREE = 512
N_TILES = D // N_FREE   # 4
KG = 4                  # k-slabs per chunk
JG = K_TILES // KG      # 4 chunks per n-tile
M_HEAD = 4              # m-tiles computed during the W stream (stage 1)
M_SLAB = 256            # tokens per stage-2 slab (two m-tiles)
N_SLABS = (M_SHARD - M_HEAD * P) // M_SLAB  # 6

_CACHE = {}


def _build_nc():
    import concourse.tile as tile
    from concourse import bacc, mybir

    nc = bacc.Bacc(None, target_bir_lowering=False,
                   enable_partition_id=False)
    f32 = mybir.dt.float32
    bf16 = mybir.dt.bfloat16

    # DRAM I/O (packed layouts: contiguous >=1KB runs per partition).
    xp = nc.dram_tensor("xp", [JG, P, KG, M_HEAD * P], bf16, kind="ExternalInput")
    xq = nc.dram_tensor("xq", [N_SLABS, JG, P, KG, M_SLAB], bf16,
                        kind="ExternalInput")
    wt = nc.dram_tensor("wt", [N_TILES, JG, P, KG, N_FREE], bf16,
                        kind="ExternalInput")
    bias = nc.dram_tensor("bias", [P, D], f32, kind="ExternalInput")
    out = nc.dram_tensor("out", [M_SHARD, D], f32, kind="ExternalOutput")

    with tile.TileContext(nc) as tc:
        with tc.tile_pool(name="wpool", bufs=1) as wpool, \
             tc.tile_pool(name="xppool", bufs=1) as xppool, \
             tc.tile_pool(name="xqpool", bufs=N_SLABS) as xqpool, \
             tc.tile_pool(name="bpool", bufs=1) as bpool, \
             tc.tile_pool(name="opool", bufs=6) as opool, \
             tc.tile_pool(name="psum", bufs=1, space="PSUM") as psum_pool:

            wc = [[None] * JG for _ in range(N_TILES)]
            xpt = [None] * JG
            bias_t = bpool.tile([P, D], f32, name="bias_t", tag="bias_t")

            # j0 of xp and W-n0 live as 4 separate kk-piece tiles so the
            # first matmul waits on one [128,512] piece, not a whole chunk.
            w00p = [wpool.tile([P, 1, N_FREE], bf16, name=f"w00_{kk}",
                               tag=f"w0_0_{kk}") for kk in range(KG)]
            xp0p = [xppool.tile([P, 1, M_HEAD * P], bf16, name=f"xp0_{kk}",
                                tag=f"xp0_{kk}") for kk in range(KG)]

            def lhs_piece(j, kk):
                return xp0p[kk][:, 0, :] if j == 0 else xpt[j][:, kk, :]

            def rhs_piece(n, j, kk):
                if n == 0 and j == 0:
                    return w00p[kk][:, 0, :]
                return wc[n][j][:, kk, :]

            # HAM warm-up: the DMA system only trickles (~75GB/s) until
            # ~12us, so real operands can't arrive sooner. Dummy matmuls on
            # memset scratch keep the PE busy from ~8us: the 2.4GHz clock
            # gate opens ~11.5us, right before the first real matmul, which
            # then runs warm along with everything after it. Fillers ending
            # early would leave a pre-flip PE gap that delays the flip
            # (HAM watches a free-running busy window), so err late.
            scr_l = xppool.tile([P, P], bf16, name="scr_l", tag="scr_l")
            scr_r = xppool.tile([P, N_FREE], bf16, name="scr_r", tag="scr_r")
            nc.vector.memset(scr_l[:, :], 0.0)
            nc.vector.memset(scr_r[:, :], 0.0)
            dps = psum_pool.tile([P, N_FREE], f32, name="dps", tag="ps7")
            for _ in range(7):
                nc.tensor.matmul(dps[:, :], lhsT=scr_l[:, :], rhs=scr_r[:, :],
                                 start=True, stop=True)

            # --- input DMA streams (two in-order HWDGE rings) ---
            # Each ring sustains only ~200GB/s while both are active, so the
            # quarter-0 critical stream (xp + W n0, ~300GB/s demand) is
            # interleaved across BOTH rings in consumption order. All late
            # traffic (W n2/n3, xq) rides Sync so the Act ring drains by
            # ~25us for the evict out-DMAs (emitted later, program order).
            ring = {0: nc.sync, 1: nc.scalar}
            for kk in range(KG):
                nc.scalar.dma_start(out=xp0p[kk][:, 0, :],
                                    in_=xp[0, :, kk, :])
                nc.sync.dma_start(out=w00p[kk][:, 0, :],
                                  in_=wt[0, 0, :, kk, :])
            for j in range(1, JG):
                xpt[j] = xppool.tile([P, KG, M_HEAD * P], bf16,
                                     name=f"xp{j}", tag=f"xp{j}")
                ring[j % 2].dma_start(out=xpt[j][:, :, :], in_=xp[j])
                w = wpool.tile([P, KG, N_FREE], bf16, name=f"w0{j}",
                               tag=f"w0_{j}")
                ring[1 - j % 2].dma_start(out=w[:, :, :], in_=wt[0, j])
                wc[0][j] = w
            nc.scalar.dma_start(out=bias_t[:, :], in_=bias[:, :])
            for j in range(JG):
                w = wpool.tile([P, KG, N_FREE], bf16, name=f"w1{j}",
                               tag=f"w1_{j}")
                ring[j % 2].dma_start(out=w[:, :, :], in_=wt[1, j])
                wc[1][j] = w
            for n in range(2, N_TILES):
                for j in range(JG):
                    w = wpool.tile([P, KG, N_FREE], bf16, name=f"w{n}{j}",
                                   tag=f"w{n}_{j}")
                    ring[j % 2].dma_start(out=w[:, :, :], in_=wt[n, j])
                    wc[n][j] = w
            # stage-2 x slabs: Sync ring after W; all resident well before
            # they are consumed in stage 2.
            xqt = [[None] * JG for _ in range(N_SLABS)]
            for s in range(N_SLABS):
                for j in range(JG):
                    t = xqpool.tile([P, KG, M_SLAB], bf16, name=f"xq{j}",
                                    tag=f"xq{j}")
                    nc.sync.dma_start(out=t[:, :, :], in_=xq[s, j])
                    xqt[s][j] = t

            def evict(ps, m, n):
                ot = opool.tile([P, N_FREE], f32, name="ot", tag="ot")
                nc.vector.tensor_add(
                    ot[:, :], ps[:, :],
                    bias_t[:, n * N_FREE:(n + 1) * N_FREE],
                )
                nc.scalar.dma_start(
                    out=out[m * P:(m + 1) * P, n * N_FREE:(n + 1) * N_FREE],
                    in_=ot[:, :],
                )

            # Stage 1: m0..3, four n-quarters, k-outer piece chase.
            # Quarter q uses PSUM banks (q%2)*4 .. (q%2)*4+3 (ping-pong).
            for q in range(N_TILES):
                pss = [psum_pool.tile([P, N_FREE], f32, name=f"ps{q}_{m}",
                                      tag=f"ps{(q % 2) * 4 + m}")
                       for m in range(M_HEAD)]
                for j in range(JG):
                    for kk in range(KG):
                        for m in range(M_HEAD):
                            nc.tensor.matmul(
                                pss[m][:, :],
                                lhsT=lhs_piece(j, kk)[:, m * P:(m + 1) * P],
                                rhs=rhs_piece(q, j, kk),
                                start=(j == 0 and kk == 0),
                                stop=(j == JG - 1 and kk == KG - 1),
                            )
                for m in range(M_HEAD):
                    evict(pss[m], m, q)

            # Stage 2: m4..15, k-inner against resident W/x.
            cnt = 0
            for s in range(N_SLABS):
                for mi in range(M_SLAB // P):
                    m = M_HEAD + s * (M_SLAB // P) + mi
                    for n in range(N_TILES):
                        ps = psum_pool.tile([P, N_FREE], f32, name="ps2",
                                            tag=f"ps{cnt % 8}")
                        cnt += 1
                        for k in range(K_TILES):
                            nc.tensor.matmul(
                                ps[:, :],
                                lhsT=xqt[s][k // KG][:, k % KG,
                                                     mi * P:(mi + 1) * P],
                                rhs=rhs_piece(n, k // KG, k % KG),
                                start=(k == 0),
                                stop=(k == K_TILES - 1),
                            )
                        evict(ps, m, n)

    nc.compile()
    return nc


def _get_nc():
    if "nc" not in _CACHE:
        _CACHE["nc"] = _build_nc()
    return _CACHE["nc"]


def _ensure_ntff_hook():
    """Register the axon NTFF profile hook (the image's antenv lacks
    axon_hooks; recreate it and wire the ctypes hook from trn_boot)."""
    import types

    try:
        from antenv.axon_hooks import get_axon_ntff_profile_hook  # noqa: F401
        return
    except ImportError:
        pass
    try:
        import antenv
        from trn_agent_boot.trn_boot import _ntff_profile_via_ctypes

        mod = types.ModuleType("antenv.axon_hooks")
        _state = {"hook": None}
        mod.set_axon_ntff_profile_hook = lambda h: _state.__setitem__("hook", h)
        mod.get_axon_ntff_profile_hook = lambda: _state["hook"]
        sys.modules["antenv.axon_hooks"] = mod
        antenv.axon_hooks = mod
        mod.set_axon_ntff_profile_hook(
            _ntff_profile_via_ctypes("/opt/axon/libaxon_pjrt.so")
        )
        # avoid the S3 artifact upload in the trace path
        import concourse.bass_utils as bu

        bu.upload_artifacts = lambda tmpdir: tmpdir
    except Exception as e:  # profiling is best-effort
        print(f"NTFF hook setup failed: {e}", file=sys.stderr)


def kernel(x, Wg, bg, We, be):
    import ml_dtypes
    from concourse.bass_utils import run_bass_kernel_spmd

    bfloat16 = ml_dtypes.bfloat16

    x = np.asarray(x, dtype=np.float32)
    Wg = np.asarray(Wg, dtype=np.float32)
    bg = np.asarray(bg, dtype=np.float32)
    We = np.asarray(We, dtype=np.float32)
    be = np.asarray(be, dtype=np.float32)

    # Row-0 gating on host (16K FLOPs): softmax over 8 logits, top-2.
    logits = x[0].astype(np.float64) @ Wg.astype(np.float64).T + bg.astype(
        np.float64
    )
    probs = np.exp(logits - logits.max())
    probs /= probs.sum()
    idx = np.argsort(-probs, kind="stable")[:TOPK]
    w0 = probs[idx]

    Wc = w0[0] * We[idx[0]].astype(np.float64) + w0[1] * We[idx[1]].astype(
        np.float64
    )
    bc = w0[0] * be[idx[0]].astype(np.float64) + w0[1] * be[idx[1]].astype(
        np.float64
    )
    WcT = np.ascontiguousarray(Wc.T).astype(np.float32)  # [d, o]
    # [n, j, p, kk, f]: d = (j kk p), o = (n f)
    wt = np.ascontiguousarray(
        WcT.reshape(JG, KG, P, N_TILES, N_FREE).transpose(3, 0, 2, 1, 4)
    ).astype(bfloat16)
    bias = np.ascontiguousarray(
        np.broadcast_to(bc.astype(np.float32), (P, D))
    )

    nc = _get_nc()
    in_maps = []
    mh = M_HEAD * P
    for c in range(N_CORES):
        xsh = x[c * M_SHARD:(c + 1) * M_SHARD]           # [m, d]
        xT = np.ascontiguousarray(xsh.T)                 # [d, m]
        x5 = xT.reshape(JG, KG, P, M_SHARD)              # [j, kk, p, m]
        # head tokens packed [j, p, kk, m]
        xph = np.ascontiguousarray(
            x5[:, :, :, :mh].transpose(0, 2, 1, 3)
        ).astype(bfloat16)
        # stage-2 slabs packed [s, j, p, kk, m]
        xqh = np.ascontiguousarray(
            x5[:, :, :, mh:].reshape(JG, KG, P, N_SLABS, M_SLAB)
            .transpose(3, 0, 2, 1, 4)
        ).astype(bfloat16)
        in_maps.append({"xp": xph, "xq": xqh, "wt": wt, "bias": bias})

    trace = bool(int(os.environ.get("KERNEL_TRACE", "0")))
    tmpdir = None
    if trace:
        import tempfile

        _ensure_ntff_hook()
        tmpdir = tempfile.mkdtemp(prefix="moe_trace_")
        _CACHE["last_tmpdir"] = tmpdir
    res = run_bass_kernel_spmd(
        nc, in_maps, core_ids=list(range(N_CORES)), trace=trace, tmpdir=tmpdir
    )
    _CACHE["last_results"] = res

    return np.concatenate(
        [res.results[c]["out"] for c in range(N_CORES)], axis=0
    )


# revision 16
# speedup vs baseline: 1.0084x; 1.0084x over previous
"""MoE routing kernel for TRN2 (8 NeuronCores).

The reference MoE applies row 0's top-2 expert choice (indices and softmax
weights) to the entire batch, so the whole module collapses to

    out = x @ (w0*We[i0] + w1*We[i1]).T + (w0*be[i0] + w1*be[i1])

a single [16384,2048] @ [2048,2048] matmul with bias. Host does the tiny
row-0 gating and combines the two selected experts; the device runs the
matmul data-parallel over tokens (2048 tokens per core, no collectives).

v3 (bf16, dual-ring chase): x and W are cast to bf16 on host (fp32 PSUM
accumulation keeps rel err ~2e-3, far inside the 2e-2 gate), halving HBM
input traffic so the whole x shard is SBUF-resident before stage 2.
HWDGE issue cost is ~600ns per dma_start regardless of size, so inputs
move as few large chunks; only the first j-chunk of xp/W is split into
[128,512] kk-pieces (own tiles => slice-granular deps) to start the PE
~1.5us after the fixed ~7us NEFF preamble. xp and early xq ride the
Activation ring in parallel with W on the Sync ring to double issue
throughput during the quarter-0 chase. Schedule:
  Stage 1: m0..3 run k-outer in four n-quarters (4 PSUM banks each,
           ping-pong between bank sets), chasing the W stream.
  Stage 2: m4..15 run k-inner against resident W/x, banks rotating mod 8.
Evictions (PSUM + bias -> SBUF -> HBM) trail on DVE/Activation queues.
"""

import os
import sys

import numpy as np

if "/opt/trn_rl_repo" not in sys.path:
    sys.path.insert(0, "/opt/trn_rl_repo")

N, D, E, TOPK = 16384, 2048, 8, 2
N_CORES = 8
P = 128
M_SHARD = N // N_CORES  # 2048 tokens per core
K_TILES = D // P        # 16 contraction slabs
M_TILES = M_SHARD // P  # 16
N_FREE = 512
N_TILES = D // N_FREE   # 4
KG = 4                  # k-slabs per chunk
JG = K_TILES // KG      # 4 chunks per n-tile
M_HEAD = 4              # m-tiles computed during the W stream (stage 1)
M_SLAB = 256            # tokens per stage-2 slab (two m-tiles)
N_SLABS = (M_SHARD - M_HEAD * P) // M_SLAB  # 6

_CACHE = {}


def _build_nc():
    import concourse.tile as tile
    from concourse import bacc, mybir

    nc = bacc.Bacc(None, target_bir_lowering=False)
    f32 = mybir.dt.float32
    bf16 = mybir.dt.bfloat16

    # DRAM I/O (packed layouts: contiguous >=1KB runs per partition).
    xp = nc.dram_tensor("xp", [JG, P, KG, M_HEAD * P], bf16, kind="ExternalInput")
    xq = nc.dram_tensor("xq", [N_SLABS, JG, P, KG, M_SLAB], bf16,
                        kind="ExternalInput")
    wt = nc.dram_tensor("wt", [N_TILES, JG, P, KG, N_FREE], bf16,
                        kind="ExternalInput")
    bias = nc.dram_tensor("bias", [P, D], f32, kind="ExternalInput")
    out = nc.dram_tensor("out", [M_SHARD, D], f32, kind="ExternalOutput")

    with tile.TileContext(nc) as tc:
        with tc.tile_pool(name="wpool", bufs=1) as wpool, \
             tc.tile_pool(name="xppool", bufs=1) as xppool, \
             tc.tile_pool(name="xqpool", bufs=N_SLABS) as xqpool, \
             tc.tile_pool(name="bpool", bufs=1) as bpool, \
             tc.tile_pool(name="opool", bufs=6) as opool, \
             tc.tile_pool(name="psum", bufs=1, space="PSUM") as psum_pool:

            wc = [[None] * JG for _ in range(N_TILES)]
            xpt = [None] * JG
            bias_t = bpool.tile([P, D], f32, name="bias_t", tag="bias_t")

            # j0 of xp and W-n0 live as 4 separate kk-piece tiles so the
            # first matmul waits on one [128,512] piece, not a whole chunk.
            w00p = [wpool.tile([P, 1, N_FREE], bf16, name=f"w00_{kk}",
                               tag=f"w0_0_{kk}") for kk in range(KG)]
            xp0p = [xppool.tile([P, 1, M_HEAD * P], bf16, name=f"xp0_{kk}",
                                tag=f"xp0_{kk}") for kk in range(KG)]

            def lhs_piece(j, kk):
                return xp0p[kk][:, 0, :] if j == 0 else xpt[j][:, kk, :]

            def rhs_piece(n, j, kk):
                if n == 0 and j == 0:
                    return w00p[kk][:, 0, :]
                return wc[n][j][:, kk, :]

            # HAM warm-up: the DMA system only trickles (~75GB/s) until
            # ~12us, so real operands can't arrive sooner. Dummy matmuls on
            # memset scratch keep the PE busy from ~8us: the 2.4GHz clock
            # gate opens ~11.5us, right before the first real matmul, which
            # then runs warm along with everything after it. Fillers ending
            # early would leave a pre-flip PE gap that delays the flip
            # (HAM watches a free-running busy window), so err late.
            scr_l = xppool.tile([P, P], bf16, name="scr_l", tag="scr_l")
            scr_r = xppool.tile([P, N_FREE], bf16, name="scr_r", tag="scr_r")
            nc.vector.memset(scr_l[:, :], 0.0)
            nc.vector.memset(scr_r[:, :], 0.0)
            dps = psum_pool.tile([P, N_FREE], f32, name="dps", tag="ps7")
            for _ in range(7):
                nc.tensor.matmul(dps[:, :], lhsT=scr_l[:, :], rhs=scr_r[:, :],
                                 start=True, stop=True)

            # --- input DMA streams (two in-order HWDGE rings) ---
            # Each ring sustains only ~200GB/s while both are active, so the
            # quarter-0 critical stream (xp + W n0, ~300GB/s demand) is
            # interleaved across BOTH rings in consumption order. All late
            # traffic (W n2/n3, xq) rides Sync so the Act ring drains by
            # ~25us for the evict out-DMAs (emitted later, program order).
            ring = {0: nc.sync, 1: nc.scalar}
            for kk in range(KG):
                nc.scalar.dma_start(out=xp0p[kk][:, 0, :],
                                    in_=xp[0, :, kk, :])
                nc.sync.dma_start(out=w00p[kk][:, 0, :],
                                  in_=wt[0, 0, :, kk, :])
            for j in range(1, JG):
                xpt[j] = xppool.tile([P, KG, M_HEAD * P], bf16,
                                     name=f"xp{j}", tag=f"xp{j}")
                ring[j % 2].dma_start(out=xpt[j][:, :, :], in_=xp[j])
                w = wpool.tile([P, KG, N_FREE], bf16, name=f"w0{j}",
                               tag=f"w0_{j}")
                ring[1 - j % 2].dma_start(out=w[:, :, :], in_=wt[0, j])
                wc[0][j] = w
            nc.scalar.dma_start(out=bias_t[:, :], in_=bias[:, :])
            for j in range(JG):
                w = wpool.tile([P, KG, N_FREE], bf16, name=f"w1{j}",
                               tag=f"w1_{j}")
                ring[j % 2].dma_start(out=w[:, :, :], in_=wt[1, j])
                wc[1][j] = w
            for n in range(2, N_TILES):
                for j in range(JG):
                    w = wpool.tile([P, KG, N_FREE], bf16, name=f"w{n}{j}",
                                   tag=f"w{n}_{j}")
                    ring[j % 2].dma_start(out=w[:, :, :], in_=wt[n, j])
                    wc[n][j] = w
            # stage-2 x slabs: Sync ring after W; all resident well before
            # they are consumed in stage 2.
            xqt = [[None] * JG for _ in range(N_SLABS)]
            for s in range(N_SLABS):
                for j in range(JG):
                    t = xqpool.tile([P, KG, M_SLAB], bf16, name=f"xq{j}",
                                    tag=f"xq{j}")
                    nc.sync.dma_start(out=t[:, :, :], in_=xq[s, j])
                    xqt[s][j] = t

            def evict(ps, m, n):
                ot = opool.tile([P, N_FREE], f32, name="ot", tag="ot")
                nc.vector.tensor_add(
                    ot[:, :], ps[:, :],
                    bias_t[:, n * N_FREE:(n + 1) * N_FREE],
                )
                nc.scalar.dma_start(
                    out=out[m * P:(m + 1) * P, n * N_FREE:(n + 1) * N_FREE],
                    in_=ot[:, :],
                )

            # Stage 1: m0..3, four n-quarters, k-outer piece chase.
            # Quarter q uses PSUM banks (q%2)*4 .. (q%2)*4+3 (ping-pong).
            for q in range(N_TILES):
                pss = [psum_pool.tile([P, N_FREE], f32, name=f"ps{q}_{m}",
                                      tag=f"ps{(q % 2) * 4 + m}")
                       for m in range(M_HEAD)]
                for j in range(JG):
                    if q == 0 and j == 1:
                        # The DMA system ramps slowly (~75GB/s) until ~12us;
                        # the resulting supply debt surfaces as a ~2us PE
                        # stall here waiting for the j1 chunks. Burn it on
                        # fillers instead so the PE (and HAM) stay busy.
                        for _ in range(9):
                            nc.tensor.matmul(dps[:, :], lhsT=scr_l[:, :],
                                             rhs=scr_r[:, :],
                                             start=True, stop=True)
                    for kk in range(KG):
                        for m in range(M_HEAD):
                            nc.tensor.matmul(
                                pss[m][:, :],
                                lhsT=lhs_piece(j, kk)[:, m * P:(m + 1) * P],
                                rhs=rhs_piece(q, j, kk),
                                start=(j == 0 and kk == 0),
                                stop=(j == JG - 1 and kk == KG - 1),
                            )
                for m in range(M_HEAD):
                    evict(pss[m], m, q)

            # Stage 2: m4..15, k-inner against resident W/x.
            cnt = 0
            for s in range(N_SLABS):
                for mi in range(M_SLAB // P):
                    m = M_HEAD + s * (M_SLAB // P) + mi
                    for n in range(N_TILES):
                        ps = psum_pool.tile([P, N_FREE], f32, name="ps2",
                                            tag=f"ps{cnt % 8}")
                        cnt += 1
                        for k in range(K_TILES):
                            nc.tensor.matmul(
                                ps[:, :],
                                lhsT=xqt[s][k // KG][:, k % KG,
                                                     mi * P:(mi + 1) * P],
                                rhs=rhs_piece(n, k // KG, k % KG),
                                start=(k == 0),
                                stop=(k == K_TILES - 1),
                            )
                        evict(ps, m, n)

    nc.compile()
    return nc


def _get_nc():
    if "nc" not in _CACHE:
        _CACHE["nc"] = _build_nc()
    return _CACHE["nc"]


def _ensure_ntff_hook():
    """Register the axon NTFF profile hook (the image's antenv lacks
    axon_hooks; recreate it and wire the ctypes hook from trn_boot)."""
    import types

    try:
        from antenv.axon_hooks import get_axon_ntff_profile_hook  # noqa: F401
        return
    except ImportError:
        pass
    try:
        import antenv
        from trn_agent_boot.trn_boot import _ntff_profile_via_ctypes

        mod = types.ModuleType("antenv.axon_hooks")
        _state = {"hook": None}
        mod.set_axon_ntff_profile_hook = lambda h: _state.__setitem__("hook", h)
        mod.get_axon_ntff_profile_hook = lambda: _state["hook"]
        sys.modules["antenv.axon_hooks"] = mod
        antenv.axon_hooks = mod
        mod.set_axon_ntff_profile_hook(
            _ntff_profile_via_ctypes("/opt/axon/libaxon_pjrt.so")
        )
        # avoid the S3 artifact upload in the trace path
        import concourse.bass_utils as bu

        bu.upload_artifacts = lambda tmpdir: tmpdir
    except Exception as e:  # profiling is best-effort
        print(f"NTFF hook setup failed: {e}", file=sys.stderr)


def kernel(x, Wg, bg, We, be):
    import ml_dtypes
    from concourse.bass_utils import run_bass_kernel_spmd

    bfloat16 = ml_dtypes.bfloat16

    x = np.asarray(x, dtype=np.float32)
    Wg = np.asarray(Wg, dtype=np.float32)
    bg = np.asarray(bg, dtype=np.float32)
    We = np.asarray(We, dtype=np.float32)
    be = np.asarray(be, dtype=np.float32)

    # Row-0 gating on host (16K FLOPs): softmax over 8 logits, top-2.
    logits = x[0].astype(np.float64) @ Wg.astype(np.float64).T + bg.astype(
        np.float64
    )
    probs = np.exp(logits - logits.max())
    probs /= probs.sum()
    idx = np.argsort(-probs, kind="stable")[:TOPK]
    w0 = probs[idx]

    Wc = w0[0] * We[idx[0]].astype(np.float64) + w0[1] * We[idx[1]].astype(
        np.float64
    )
    bc = w0[0] * be[idx[0]].astype(np.float64) + w0[1] * be[idx[1]].astype(
        np.float64
    )
    WcT = np.ascontiguousarray(Wc.T).astype(np.float32)  # [d, o]
    # [n, j, p, kk, f]: d = (j kk p), o = (n f)
    wt = np.ascontiguousarray(
        WcT.reshape(JG, KG, P, N_TILES, N_FREE).transpose(3, 0, 2, 1, 4)
    ).astype(bfloat16)
    bias = np.ascontiguousarray(
        np.broadcast_to(bc.astype(np.float32), (P, D))
    )

    nc = _get_nc()
    in_maps = []
    mh = M_HEAD * P
    for c in range(N_CORES):
        xsh = x[c * M_SHARD:(c + 1) * M_SHARD]           # [m, d]
        xT = np.ascontiguousarray(xsh.T)                 # [d, m]
        x5 = xT.reshape(JG, KG, P, M_SHARD)              # [j, kk, p, m]
        # head tokens packed [j, p, kk, m]
        xph = np.ascontiguousarray(
            x5[:, :, :, :mh].transpose(0, 2, 1, 3)
        ).astype(bfloat16)
        # stage-2 slabs packed [s, j, p, kk, m]
        xqh = np.ascontiguousarray(
            x5[:, :, :, mh:].reshape(JG, KG, P, N_SLABS, M_SLAB)
            .transpose(3, 0, 2, 1, 4)
        ).astype(bfloat16)
        in_maps.append({"xp": xph, "xq": xqh, "wt": wt, "bias": bias})

    trace = bool(int(os.environ.get("KERNEL_TRACE", "0")))
    tmpdir = None
    if trace:
        import tempfile

        _ensure_ntff_hook()
        tmpdir = tempfile.mkdtemp(prefix="moe_trace_")
        _CACHE["last_tmpdir"] = tmpdir
    res = run_bass_kernel_spmd(
        nc, in_maps, core_ids=list(range(N_CORES)), trace=trace, tmpdir=tmpdir
    )
    _CACHE["last_results"] = res

    return np.concatenate(
        [res.results[c]["out"] for c in range(N_CORES)], axis=0
    )
